# revision 7
# baseline (speedup 1.0000x reference)
"""Trainium2 Bass kernel for nn_ActorCritic (3-layer edge-GNN, qconv stack).

Strategy (8 NeuronCores), v2 "k-row" design:
  - Nodes dst-sharded 8 ways; per core, own nodes are PERMUTED so that each
    128-node dst block groups nodes of similar in-degree.
  - Edges of a block are laid out in "k-rows": tile k of block b holds the
    k-th in-edge of each of the block's 128 nodes (partition j = node j).
    The segment-MEAN then needs NO one-hot selection matrices: for every
    tile,  psum_s[f, d] += t_tile.T @ D_b  with D_b = diag(1/deg) — a plain
    PSUM-accumulating matmul whose moving operand is a tiny constant.
  - Per layer l>=1: u_l = h_{l-1} @ W1_l[:, :F].T per-core, AllGathered into
    DRAM tables (lo/hi split for collective overlap); per-edge work is
      t_e = prelu(u_l[src_e] + v_e),  v_e = w_e @ W1_l[:, F:].T (host),
    with u gathered by dma_gather, the add on DVE and prelu alternating
    DVE (scalar_tensor_tensor max(x, .01x)) / ACT — all at call granularity
    (24 tiles = 3072 edges per op) to amortize fixed engine overheads.
  - Layer 0 uses a one-hot trick: t0 = prelu(S0.T @ rhs0) with host-built
    S0 = [onehot(gate_type[src]); w_e] and rhs0 = [emb @ W1a.T; W1w.T].
  - Combine stays feat-major end-to-end: h_l = relu(W2a@h + W2b@h_N + b2).
Host precomputes all index/layout arrays; entry point kernel(**inputs).
"""

import hashlib
import numpy as np
import ml_dtypes

import concourse.bass as bass
import concourse.bacc as bacc
import concourse.tile as tile
import concourse.mybir as mybir
from concourse.bass_utils import run_bass_kernel_spmd

BF16 = ml_dtypes.bfloat16
F32 = np.float32

N_NODES = 50000
N_EDGES = 800000
F0 = 32           # input feats (num gate types)
H = 128           # hidden dim
NC = 8            # cores
NPC = N_NODES // NC      # 6250 nodes per core
NB = 49                  # dst 128-blocks per core (49*128 = 6272)
NBW = NB * 128           # padded own-node count
SPLIT = 3072             # lo region rows per rank (block-aligned, 24 blocks)
LOB = SPLIT // 128       # 24 lo blocks
HIW = NBW - SPLIT        # 3200 hi rows per rank (25 blocks)
ZLO = NC * SPLIT         # zero-row index in lo table
ZHI = NC * HIW           # zero-row index in hi table
TILE = 128
CALL_TILES = 24          # tiles per dma_gather call (3072 edge slots)
PRELU_ALPHA = 0.01

PROFILE = False          # set True (e.g. from test.py) to capture HW timing
TRACE_DIR = None         # optional dir for NTFF/perfetto artifacts
LAST_EXEC_NS = None

_cache = {}


# ----------------------------------------------------------------- host prep

def _schedule_and_arrays(gate_type, edge_src, edge_dst, edge_w,
                         emb, W1_0, W2_0, b2_0, W1_rest, W2_rest, b2_rest):
    src_all = np.asarray(edge_src).astype(np.int64)
    dst_all = np.asarray(edge_dst).astype(np.int64)
    gt_all = np.asarray(gate_type).astype(np.int64)
    w_all = np.asarray(edge_w).astype(np.float32)
    core_of = dst_all // NPC

    pc = []
    for c in range(NC):
        m = core_of == c
        pc.append((src_all[m], dst_all[m] - c * NPC, w_all[m]))

    # --- step 1: provisional per-core permutation by total in-degree
    pos0 = np.empty((NC, NPC), np.int64)
    for c in range(NC):
        tot = np.bincount(pc[c][1], minlength=NPC)
        order0 = np.argsort(-tot, kind="stable")
        pos0[c, order0] = np.arange(NPC)

    # --- step 2: classify edges under perm0, refine perm by max(lo, hi)
    pos1 = np.empty((NC, NPC), np.int64)
    order1 = np.empty((NC, NPC), np.int64)
    for c in range(NC):
        s, dl, _ = pc[c]
        lo0 = pos0[s // NPC, s % NPC] < SPLIT
        lo_deg = np.bincount(dl[lo0], minlength=NPC)
        hi_deg = np.bincount(dl[~lo0], minlength=NPC)
        o1 = np.argsort(-np.maximum(lo_deg, hi_deg), kind="stable")
        order1[c] = o1
        pos1[c, o1] = np.arange(NPC)

    # --- step 3: final classification + per-block tile counts (shared SPMD)
    blkmax = np.zeros((NC, 2, NB), np.int64)
    finals = []
    for c in range(NC):
        s, dl, w = pc[c]
        lo_e = pos1[s // NPC, s % NPC] < SPLIT
        q = pos1[c, dl]                       # permuted position of dst
        ldeg = np.bincount(q[lo_e], minlength=NBW)
        hdeg = np.bincount(q[~lo_e], minlength=NBW)
        blkmax[c, 0] = ldeg.reshape(NB, TILE).max(1)
        blkmax[c, 1] = hdeg.reshape(NB, TILE).max(1)
        finals.append((lo_e, q))
    ntiles = blkmax.max(axis=0)               # [2, NB]

    # --- tile list / sessions / calls
    tiles = []
    sess_start, sess_end = [], []
    tile_base = {}
    for p in (0, 1):
        for b in range(NB):
            nt = int(ntiles[p][b])
            if nt == 0:
                continue
            tile_base[(p, b)] = len(tiles)
            for k in range(nt):
                tiles.append((p, b))
                sess_start.append(k == 0)
                sess_end.append(k == nt - 1)
    NT = len(tiles)
    pass0_tiles = int(ntiles[0].sum())

    calls = []
    for p, lo_t, hi_t in ((0, 0, pass0_tiles), (1, pass0_tiles, NT)):
        t = lo_t
        while t < hi_t:
            t1 = min(t + CALL_TILES, hi_t)
            calls.append((p, t, t1))
            t = t1

    W1w = [np.asarray(W1_0)[:, F0:F0 + 3],
           np.asarray(W1_rest)[0][:, H:H + 3],
           np.asarray(W1_rest)[1][:, H:H + 3]]

    per_core = []
    for c in range(NC):
        s, dl, w = pc[c]
        lo_e, q = finals[c]
        # per-edge slot: tile (p, b, k), partition j = q % 128
        b = q // TILE
        j = q % TILE
        p_e = (~lo_e).astype(np.int64)
        # k = rank among edges with same (p, q)
        keyq = p_e * NBW + q
        order = np.argsort(keyq, kind="stable")
        ks = keyq[order]
        first = np.zeros(2 * NBW, np.int64)
        np.cumsum(np.bincount(ks, minlength=2 * NBW)[:-1], out=first[1:])
        krank = np.arange(len(ks)) - first[ks]
        tb = np.array([tile_base.get((int(pp), int(bb)), 0)
                       for pp in (0, 1) for bb in range(NB)]).reshape(2, NB)
        slot = (tb[p_e[order], b[order]] + krank) * TILE + j[order]

        so, wo = s[order], w[order]
        # idx12: (rank, permuted pos) of src; default = zero rows per pass
        idx12 = np.zeros(NT * TILE, np.int32)
        for p in (0, 1):
            lo_t = 0 if p == 0 else pass0_tiles
            hi_t = pass0_tiles if p == 0 else NT
            idx12[lo_t * TILE:hi_t * TILE] = ZLO if p == 0 else ZHI
        sr = so // NPC
        sp = pos1[sr, so % NPC]
        enc = np.where(sp < SPLIT, sr * SPLIT + sp,
                       sr * HIW + (sp - SPLIT))
        idx12[slot] = enc
        idx12 = idx12.astype(np.int16)

        # v1 / v2 per slot  [TILE, NT, H]
        vs = []
        for l in (1, 2):
            vfull = np.zeros((NT * TILE, H), np.float32)
            vfull[slot] = wo @ W1w[l].T
            vs.append(np.ascontiguousarray(
                vfull.reshape(NT, TILE, H).transpose(1, 0, 2)).astype(BF16))

        # s0: [35, NT, 128] one-hot(gate_type[src]) ; w rows
        s0 = np.zeros((F0 + 3, NT * TILE), np.float32)
        s0[gt_all[so], slot] = 1.0
        s0[F0:, slot] = wo.T
        s0 = np.ascontiguousarray(
            s0.reshape(F0 + 3, NT, TILE)).astype(BF16)

        # D: [128, NBW] block-diagonal inv-count (bf16)
        tot = np.bincount(dl, minlength=NPC)
        inv = np.zeros(NBW, np.float32)
        inv[:NPC] = 1.0 / np.maximum(tot[order1[c]], 1.0)
        D = np.zeros((TILE, NBW), np.float32)
        D[np.arange(NBW) % TILE, np.arange(NBW)] = inv

        h0_full = np.asarray(emb)[gt_all]            # [N, F0]
        h0T = np.zeros((F0, NBW), np.float32)
        h0T[:, :NPC] = h0_full[c * NPC:(c + 1) * NPC][order1[c]].T

        def wrap(a):
            outs = []
            for (_, t0, t1) in calls:
                seg = a[t0 * TILE:t1 * TILE].reshape(-1, 16).T
                outs.append(np.tile(seg, (8, 1)))
            return np.ascontiguousarray(np.concatenate(outs, axis=1))

        per_core.append({
            "idx12": wrap(idx12),
            "s0": s0,
            "v1": vs[0], "v2": vs[1],
            "dmat": np.ascontiguousarray(D).astype(BF16),
            "h0T": h0T.astype(BF16),
        })

    # shared weights
    table0 = np.asarray(emb) @ np.asarray(W1_0)[:, :F0].T   # [32,128]
    rhs0 = np.concatenate([table0, W1w[0].T], axis=0).astype(BF16)  # [35,128]
    w1ht = np.stack([np.asarray(W1_rest)[0][:, :H].T,
                     np.asarray(W1_rest)[1][:, :H].T]).astype(BF16)
    w2at0 = np.asarray(W2_0)[:, :F0].T.astype(BF16)        # [32, 128]
    w2bt0 = np.asarray(W2_0)[:, F0:].T.astype(BF16)        # [128, 128]
    w2at12 = np.stack([np.asarray(W2_rest)[0][:, :H].T,
                       np.asarray(W2_rest)[1][:, :H].T]).astype(BF16)
    w2bt12 = np.stack([np.asarray(W2_rest)[0][:, H:].T,
                       np.asarray(W2_rest)[1][:, H:].T]).astype(BF16)
    b2t = np.stack([np.asarray(b2_0),
                    np.asarray(b2_rest)[0],
                    np.asarray(b2_rest)[1]]).T.astype(F32)  # [128, 3]
    ident = np.eye(TILE, dtype=np.float32).astype(BF16)

    shared = {
        "rhs0": rhs0, "w1ht": w1ht,
        "w2at0": w2at0, "w2bt0": w2bt0,
        "w2at12": w2at12, "w2bt12": w2bt12,
        "b2t": b2t, "ident": ident,
    }
    for m in per_core:
        m.update(shared)

    sched = {
        "NT": NT, "tiles": tiles, "sess_start": sess_start,
        "sess_end": sess_end, "calls": calls,
        "ntiles": ntiles, "tile_base": tile_base,
        "pass0_tiles": pass0_tiles,
    }
    return sched, per_core, order1


# ------------------------------------------------------------------- codegen

def _emit_prologue(env):
    nc, pools, P, dt = env["nc"], env["pools"], env["P"], env["dt"]
    NT = env["sched"]["NT"]
    constp = pools["const"]

    def load_const(name, shape, dtyp):
        t_ = constp.tile(shape, dtyp, tag=name)
        nc.sync.dma_start(t_[:], P[name][:])
        return t_

    env["ident_sb"] = load_const("ident", [128, 128], dt.bfloat16)
    env["b2_sb"] = load_const("b2t", [H, 3], dt.float32)
    env["rhs0_sb"] = load_const("rhs0", [F0 + 3, H], dt.bfloat16)
    env["d_sb"] = load_const("dmat", [128, NBW], dt.bfloat16)
    env["w2at0_sb"] = load_const("w2at0", [F0, H], dt.bfloat16)
    env["w2bt0_sb"] = load_const("w2bt0", [H, H], dt.bfloat16)
    for nm in ("w1ht", "w2at12", "w2bt12"):
        t_ = constp.tile([H, 2, H], dt.bfloat16, tag=nm)
        nc.sync.dma_start(t_[:], P[nm].ap().rearrange("a k m -> k a m"))
        env[nm + "_sb"] = t_
    idx12_sb = pools["idxr"].tile([128, NT * 8], dt.int16)
    nc.sync.dma_start(idx12_sb[:], P["idx12"][:])
    env["idx12_sb"] = idx12_sb
    h0T_sb = constp.tile([F0, NBW], dt.bfloat16, tag="h0T")
    nc.sync.dma_start(h0T_sb[:], P["h0T"][:])
    env["h_prevT"] = h0T_sb
    # zero rows at the tail of the shared u tables (gather target for padding)
    zr = constp.tile([128, H], dt.bfloat16, tag="zr")
    nc.vector.memset(zr[:], 0.0)
    nc.sync.dma_start(env["u_table_lo"][ZLO:ZLO + 128, :], zr[:])
    nc.sync.dma_start(env["u_table_hi"][ZHI:ZHI + 128, :], zr[:])
    # idx column offsets per call (wrapped layout)
    call_cols = []
    off = 0
    for (_, t0, t1) in env["sched"]["calls"]:
        ct = t1 - t0
        call_cols.append(off)
        off += ct * 8
    env["call_cols"] = call_cols


def _emit_allgather(env, part):
    nc = env["nc"]
    if part == 0:
        ins = env["u_own_lo"][:, :].opt()
        outs = env["u_table_lo"][0:ZLO, :].opt()
    else:
        ins = env["u_own_hi"][:, :].opt()
        outs = env["u_table_hi"][0:ZHI, :].opt()
    nc.gpsimd.collective_compute(
        "AllGather", mybir.AluOpType.bypass,
        replica_groups=[list(range(NC))],
        ins=[ins], outs=[outs],
    )


def _acc_mm(env, l, gt, tt_ap, state):
    """PSUM-accumulating segment-mean matmul for one 128-edge tile."""
    nc, pools, dt = env["nc"], env["pools"], env["dt"]
    sched = env["sched"]
    pss, b = sched["tiles"][gt]
    if sched["sess_start"][gt]:
        state["ps"] = pools["ps"].tile([128, 128], dt.float32, name="ps")
    nc.tensor.matmul(state["ps"][:], tt_ap, env["d_sb"][:, b * 128:(b + 1) * 128],
                     start=bool(sched["sess_start"][gt]),
                     stop=bool(sched["sess_end"][gt]))
    if sched["sess_end"][gt]:
        cols = slice(b * 128, (b + 1) * 128)
        if pss == 0:
            nc.vector.tensor_copy(env["s_lo"][:, cols], state["ps"][:])
        else:
            nc.vector.tensor_tensor(
                out=env["hn"][:, cols], in0=state["ps"][:],
                in1=env["s_lo"][:, cols], op=mybir.AluOpType.add)


def _emit_call(env, l, ci, state):
    nc, pools, dt = env["nc"], env["pools"], env["dt"]
    pss, t0, t1 = env["sched"]["calls"][ci]
    ct = t1 - t0
    coff = env["call_cols"][ci]
    if l == 0:
        s0c = pools["s0p"].tile([F0 + 3, CALL_TILES, TILE], dt.bfloat16, tag="s0")
        nc.sync.dma_start(s0c[:, :ct, :], env["P"]["s0"][:, t0:t1, :])
        for j4 in range(0, ct, 4):
            nj = min(4, ct - j4)
            pt0 = pools["pt0"].tile([128, 4, TILE], dt.float32, tag="pt0",
                                    name="pt0")
            for jj in range(nj):
                nc.tensor.matmul(pt0[:, jj, :], s0c[:, j4 + jj, :],
                                 env["rhs0_sb"][:], start=True, stop=True)
            tt = pools["t0p"].tile([128, 4, H], dt.bfloat16, tag="t0")
            nc.scalar.activation(tt[:, :nj, :], pt0[:, :nj, :],
                                 mybir.ActivationFunctionType.Prelu,
                                 alpha=PRELU_ALPHA)
            for jj in range(nj):
                _acc_mm(env, l, t0 + j4 + jj, tt[:, jj, :], state)
    else:
        idx_ap = env["idx12_sb"][:, coff:coff + ct * 8]
        view = env["u_table_lo"][:] if pss == 0 else env["u_table_hi"][:]
        g = pools["g"].tile([128, CALL_TILES, H], dt.bfloat16, tag="g")
        nc.gpsimd.dma_gather(
            out_ap=g[:, :ct, :], in_ap=view, idxs_ap=idx_ap,
            num_idxs=ct * 128, num_idxs_reg=ct * 128, elem_size=H,
            single_packet=False, queue_num=ci % 4,
        )
        vsl = pools["v"].tile([128, CALL_TILES, H], dt.bfloat16, tag="v")
        nc.sync.dma_start(vsl[:, :ct, :], env["P"][f"v{l}"][:, t0:t1, :])
        y = pools["y"].tile([128, CALL_TILES, H], dt.bfloat16, tag="y")
        nc.vector.tensor_tensor(out=y[:, :ct, :], in0=g[:, :ct, :],
                                in1=vsl[:, :ct, :], op=mybir.AluOpType.add)
        tt = pools["t"].tile([128, CALL_TILES, H], dt.bfloat16, tag="t")
        if ci % 2 == 0:
            nc.vector.scalar_tensor_tensor(
                out=tt[:, :ct, :], in0=y[:, :ct, :], scalar=PRELU_ALPHA,
                in1=y[:, :ct, :], op0=mybir.AluOpType.mult,
                op1=mybir.AluOpType.max)
        else:
            nc.scalar.activation(tt[:, :ct, :], y[:, :ct, :],
                                 mybir.ActivationFunctionType.Prelu,
                                 alpha=PRELU_ALPHA)
        for j in range(ct):
            _acc_mm(env, l, t0 + j, tt[:, j, :], state)


def _emit_chunk(env, l, k):
    """Epilogue for node chunk k (4 blocks): combine, and (l<2) next-layer
    u rows + store."""
    nc, pools, dt = env["nc"], env["pools"], env["dt"]
    ntiles = env["sched"]["ntiles"]
    b0 = 4 * k
    nb = min(4, NB - b0)
    ck0, ck = 512 * k, 128 * nb

    for b in range(b0, b0 + nb):
        cols = slice(b * 128, (b + 1) * 128)
        if ntiles[1][b] == 0:
            if ntiles[0][b] > 0:
                nc.vector.tensor_copy(env["hn"][:, cols], env["s_lo"][:, cols])
            else:
                nc.vector.memset(env["hn"][:, cols], 0.0)

    if l == 0:
        w2a, w2b = env["w2at0_sb"][:], env["w2bt0_sb"][:]
    else:
        w2a = env["w2at12_sb"][:, l - 1, :]
        w2b = env["w2bt12_sb"][:, l - 1, :]
    bias = env["b2_sb"][:, l:l + 1]
    pc_ = pools["p512"].tile([128, 512], dt.float32, tag="p512", name="pc_")
    nc.tensor.matmul(pc_[:, :ck], w2a, env["h_prevT"][:, ck0:ck0 + ck],
                     start=True, stop=False)
    nc.tensor.matmul(pc_[:, :ck], w2b, env["hn"][:, ck0:ck0 + ck],
                     start=False, stop=True)
    if l < 2:
        h_outT = env["h_outT"]
        nc.scalar.activation(h_outT[:, ck0:ck0 + ck], pc_[:, :ck],
                             mybir.ActivationFunctionType.Relu, bias=bias)
        # next-layer u rows for this chunk: u = W1h_{l+1} @ h_out
        pu = pools["p512"].tile([128, 512], dt.float32, tag="p512", name="pu")
        nc.tensor.matmul(pu[:, :ck], env["w1ht_sb"][:, l, :],
                         h_outT[:, ck0:ck0 + ck], start=True, stop=True)
        uTc = pools["scr"].tile([H, 512], dt.bfloat16, tag="uTc")
        nc.scalar.activation(uTc[:, :ck], pu[:, :ck],
                             mybir.ActivationFunctionType.Copy)
        u_nm = pools["unm"].tile([128, 4, H], dt.bfloat16, tag="unm")
        for j in range(nb):
            ptru = pools["ptr"].tile([128, 128], dt.bfloat16, tag="ptr",
                                     name="ptru")
            nc.tensor.transpose(ptru[:], uTc[:, j * 128:(j + 1) * 128],
                                env["ident_sb"][:])
            nc.vector.tensor_copy(u_nm[:, j, :], ptru[:])
            b = b0 + j
            if b < LOB:
                dst = env["u_own_lo"][b * 128:(b + 1) * 128, :]
            else:
                bb = b - LOB
                dst = env["u_own_hi"][bb * 128:(bb + 1) * 128, :]
            nc.sync.dma_start(dst, u_nm[:, j, :])
    else:
        oc = pools["oc"].tile([H, 512], dt.float32, tag="oc")
        nc.scalar.activation(oc[:, :ck], pc_[:, :ck],
                             mybir.ActivationFunctionType.Relu, bias=bias)
        nc.sync.dma_start(env["out_ext"][:, ck0:ck0 + ck], oc[:, :ck])


def _emit_program(env):
    _emit_prologue(env)
    sched = env["sched"]
    ncalls = len(sched["calls"])
    ntiles = sched["ntiles"]
    tile_base = sched["tile_base"]
    NCH = (NB + 3) // 4

    # per-block final tile (pass 1 preferred, else pass 0, else -1)
    final_tile = {}
    for b in range(NB):
        if ntiles[1][b] > 0:
            final_tile[b] = tile_base[(1, b)] + int(ntiles[1][b]) - 1
        elif ntiles[0][b] > 0:
            final_tile[b] = tile_base[(0, b)] + int(ntiles[0][b]) - 1
        else:
            final_tile[b] = -1
    chunk_ready = [max(final_tile[b] for b in range(4 * k, min(4 * k + 4, NB)))
                   for k in range(NCH)]

    pools, dt = env["pools"], env["dt"]
    for l in range(3):
        env["s_lo"] = pools["slo"].tile([128, NBW], dt.bfloat16, tag="slo",
                                        name=f"slo{l}")
        env["hn"] = pools["hnp"].tile([128, NBW], dt.bfloat16, tag="hn",
                                      name=f"hn{l}")
        # degenerate blocks: no lo tiles -> zero their s_lo columns
        for b in range(NB):
            if ntiles[0][b] == 0 and ntiles[1][b] > 0:
                nc_ = env["nc"]
                nc_.vector.memset(env["s_lo"][:, b * 128:(b + 1) * 128], 0.0)
        if l < 2:
            env["h_outT"] = pools["h"].tile([H, NBW], dt.bfloat16, tag="h",
                                            name=f"h{l + 1}")
        nxt = 0
        state = {}

        def emit_chunks_until(bound):
            nonlocal nxt
            while nxt < bound:
                _emit_chunk(env, l, nxt)
                nxt += 1
                if l < 2 and nxt == SPLIT // 512:
                    _emit_allgather(env, 0)

        for ci in range(ncalls):
            _emit_call(env, l, ci, state)
            t1 = sched["calls"][ci][2]
            while nxt < NCH and chunk_ready[nxt] < t1:
                emit_chunks_until(nxt + 1)
        emit_chunks_until(NCH)
        if l < 2:
            _emit_allgather(env, 1)
            env["h_prevT"] = env["h_outT"]


def _build_nc(sched):
    NT = sched["NT"]
    dt = mybir.dt

    nc = bacc.Bacc("TRN2", target_bir_lowering=False, debug=False,
                   num_devices=NC, num_swdge_queues=4)

    P = {}
    P["idx12"] = nc.dram_tensor("idx12", [128, NT * 8], dt.int16, kind="ExternalInput")
    P["s0"] = nc.dram_tensor("s0", [F0 + 3, NT, TILE], dt.bfloat16, kind="ExternalInput")
    for l in (1, 2):
        P[f"v{l}"] = nc.dram_tensor(f"v{l}", [128, NT, H], dt.bfloat16, kind="ExternalInput")
    P["dmat"] = nc.dram_tensor("dmat", [128, NBW], dt.bfloat16, kind="ExternalInput")
    P["h0T"] = nc.dram_tensor("h0T", [F0, NBW], dt.bfloat16, kind="ExternalInput")
    P["rhs0"] = nc.dram_tensor("rhs0", [F0 + 3, H], dt.bfloat16, kind="ExternalInput")
    P["w1ht"] = nc.dram_tensor("w1ht", [2, H, H], dt.bfloat16, kind="ExternalInput")
    P["w2at0"] = nc.dram_tensor("w2at0", [F0, H], dt.bfloat16, kind="ExternalInput")
    P["w2bt0"] = nc.dram_tensor("w2bt0", [H, H], dt.bfloat16, kind="ExternalInput")
    P["w2at12"] = nc.dram_tensor("w2at12", [2, H, H], dt.bfloat16, kind="ExternalInput")
    P["w2bt12"] = nc.dram_tensor("w2bt12", [2, H, H], dt.bfloat16, kind="ExternalInput")
    P["b2t"] = nc.dram_tensor("b2t", [H, 3], dt.float32, kind="ExternalInput")
    P["ident"] = nc.dram_tensor("ident", [128, 128], dt.bfloat16, kind="ExternalInput")

    out_ext = nc.dram_tensor("out", [H, NBW], dt.float32, kind="ExternalOutput")
    u_own_lo = nc.dram_tensor("u_own_lo", [SPLIT, H], dt.bfloat16)
    u_own_hi = nc.dram_tensor("u_own_hi", [HIW, H], dt.bfloat16)
    u_table_lo = nc.dram_tensor("u_table_lo", [NC * SPLIT + 128, H], dt.bfloat16,
                                addr_space="Shared")
    u_table_hi = nc.dram_tensor("u_table_hi", [NC * HIW + 128, H], dt.bfloat16,
                                addr_space="Shared")

    from contextlib import ExitStack
    with tile.TileContext(nc) as tc, ExitStack() as ctx:
        pools = {}
        for nm, bufs, space in [
            ("const", 1, "SBUF"), ("idxr", 1, "SBUF"),
            ("g", 3, "SBUF"), ("v", 3, "SBUF"), ("y", 2, "SBUF"),
            ("t", 3, "SBUF"), ("t0p", 4, "SBUF"), ("s0p", 3, "SBUF"),
            ("slo", 1, "SBUF"), ("hnp", 1, "SBUF"),
            ("h", 2, "SBUF"), ("scr", 2, "SBUF"), ("unm", 2, "SBUF"),
            ("oc", 2, "SBUF"),
            ("ps", 2, "PSUM"), ("pt0", 2, "PSUM"), ("ptr", 2, "PSUM"),
            ("p512", 2, "PSUM"),
        ]:
            pools[nm] = ctx.enter_context(tc.tile_pool(name=nm, bufs=bufs, space=space))
        env = dict(nc=nc, tc=tc, pools=pools, P=P, out_ext=out_ext,
                   u_own_lo=u_own_lo, u_own_hi=u_own_hi,
                   u_table_lo=u_table_lo, u_table_hi=u_table_hi,
                   sched=sched, dt=dt)
        _emit_program(env)

    nc.compile()
    return nc


# --------------------------------------------------------------------- entry

def kernel(gate_type, edge_src, edge_dst, edge_w, emb, W1_0, W2_0, b2_0,
           W1_rest, W2_rest, b2_rest):
    global LAST_EXEC_NS
    key = hashlib.sha1(
        np.ascontiguousarray(np.asarray(edge_dst, dtype=np.int64)).tobytes()
        + np.ascontiguousarray(np.asarray(edge_src, dtype=np.int64)).tobytes()
    ).hexdigest()

    sched, per_core, order1 = _schedule_and_arrays(
        gate_type, edge_src, edge_dst, edge_w, emb, W1_0, W2_0, b2_0,
        W1_rest, W2_rest, b2_rest)

    if key in _cache and _cache[key][1]["NT"] == sched["NT"]:
        nc = _cache[key][0]
    else:
        nc = _build_nc(sched)
        _cache.clear()
        _cache[key] = (nc, sched)

    res = run_bass_kernel_spmd(nc, per_core, core_ids=list(range(NC)),
                               trace=PROFILE, tmpdir=TRACE_DIR)
    LAST_EXEC_NS = res.exec_time_ns

    out = np.empty((N_NODES, H), np.float32)
    for c in range(NC):
        oc = res.results[c]["out"][:, :NPC].T          # permuted node rows
        out[c * NPC + order1[c]] = oc
    return out
